# revision 12
# baseline (speedup 1.0000x reference)
"""Dinov3 self-attention Bass kernel for TRN2.

Sharding: data-parallel over batch. B=8 batch elements -> 8 NeuronCores,
one full attention per core, weights replicated. No collectives.

v3 design (v2 + trace-driven fixes):
  - v2 traits kept: qT/kT/ctxT head-transposed layouts, psum-packed
    two-head score chunks with one wide exp, ones-column denominator
    trick, on-chip normalize, paced projection fillers.
  - PE warmup burst: ~36 dummy matmuls at t~2us flip the HAM clock
    gate to 8/8 before the real prologue matmuls issue (transposes
    don't register as PE activity, so without this the whole prologue
    runs at 1.2 GHz).
  - prologue is emission-interleaved: qkproj(0) chunk-halves between
    x-tile transposes, casts/evicts alternating DVE/ACT, so the PE
    never head-of-line blocks on a single elementwise queue.
  - v-bias folded into the output projection: softmax rows sum to 1,
    so ctx(v + bv) = ctx(v) + bv and  out = ctx@Wp.T + (Wp@bv + bp).
    bp' = Wp@bv + bp is computed once on the PE (12 tiny matmuls);
    the 66 per-tile v-bias matmuls + LDWEIGHTS disappear.
  - rope rotate DMAs, bias loads and sincos staging post from the
    (otherwise idle) GpSimd queue: in v2 they sat on the Sync queue
    behind DMA_TRANSPOSE posts whose semaphore waits head-of-line
    blocked them ~50us, starving DVE and stalling the PE 13us at a
    time mid-kernel.
  - sincos staged with 4 large DMAs (was 22 small ones) and 8 wide
    scatter copies (was 44).
Engines: PE matmuls; ACT exp + prologue casts/evicts; DVE everything
elementwise; GpSimd queue posts rope/bias/sincos DMAs; Sync queue
posts x/w loads, w transposes, out stores.
"""

import contextlib
import sys

import numpy as np

sys.path.insert(0, "/opt/trn_rl_repo")

import concourse.bacc as bacc
import concourse.bass as bass
import concourse.tile as tile
from concourse import mybir

S = 1374
H = 768
NH = 12
D = 64
NROT = 1369
PREFIX = S - NROT  # 5
B = 8

P = 128
NSTILE = (S + P - 1) // P  # 11 s-tiles, last has 94 rows
NOTILE = H // P  # 6 head pairs
SPAD = NSTILE * P  # 1408
IC = ((0, 512), (512, 512), (1024, 350))  # i-chunks, each <= 1 psum bank

F32 = mybir.dt.float32
BF16 = mybir.dt.bfloat16


def _stile(i):
    start = i * P
    return start, min(P, S - start)


def build_kernel(nc):
    x_ext = nc.declare_dram_parameter("hidden_states", [S, H], F32, isOutput=False)
    sin_ext = nc.declare_dram_parameter("sin", [NROT, D], F32, isOutput=False)
    cos_ext = nc.declare_dram_parameter("cos", [NROT, D], F32, isOutput=False)
    wq_ext = nc.declare_dram_parameter("Wq", [H, H], F32, isOutput=False)
    bq_ext = nc.declare_dram_parameter("bq", [H], F32, isOutput=False)
    wk_ext = nc.declare_dram_parameter("Wk", [H, H], F32, isOutput=False)
    wv_ext = nc.declare_dram_parameter("Wv", [H, H], F32, isOutput=False)
    bv_ext = nc.declare_dram_parameter("bv", [H], F32, isOutput=False)
    wp_ext = nc.declare_dram_parameter("Wp", [H, H], F32, isOutput=False)
    bp_ext = nc.declare_dram_parameter("bp", [H], F32, isOutput=False)
    out_ext = nc.declare_dram_parameter("out", [S, H], F32, isOutput=True)

    with tile.TileContext(nc) as tc:
        _body(tc, x_ext, sin_ext, cos_ext, wq_ext, bq_ext, wk_ext,
              wv_ext, bv_ext, wp_ext, bp_ext, out_ext)
    nc.compile()
    return nc


def _body(tc, x_ext, sin_ext, cos_ext, wq_ext, bq_ext, wk_ext, wv_ext,
          bv_ext, wp_ext, bp_ext, out_ext):
    nc = tc.nc

    with contextlib.ExitStack() as ctx:
        persist = ctx.enter_context(tc.tile_pool(name="persist", bufs=1))
        psum_sc = ctx.enter_context(tc.tile_pool(name="psum_sc", bufs=2, space="PSUM"))
        psum_pv = ctx.enter_context(tc.tile_pool(name="psum_pv", bufs=1, space="PSUM"))
        psum_pj = ctx.enter_context(tc.tile_pool(name="psum_pj", bufs=2, space="PSUM"))
        es_pool = ctx.enter_context(tc.tile_pool(name="es_pool", bufs=3))
        rec_pool = ctx.enter_context(tc.tile_pool(name="rec_pool", bufs=1))
        ropet = ctx.enter_context(tc.tile_pool(name="ropet", bufs=2))
        stage = ctx.enter_context(tc.tile_pool(name="stage", bufs=2))
        cstage = ctx.enter_context(tc.tile_pool(name="cstage", bufs=1))
        outst = ctx.enter_context(tc.tile_pool(name="outst", bufs=2))

        xT = persist.tile([P, NOTILE, SPAD], BF16)     # xT[p, t, s] = x[s, 128t+p]
        qT = persist.tile([P, NOTILE, SPAD], BF16)     # roped q, [(hh,d), pt, s]
        kT = persist.tile([P, NOTILE, SPAD], BF16)
        ctxT = persist.tile([P, NOTILE, SPAD], BF16)   # normalized ctx^T
        # per head: 64 ones columns + 64 v columns, so every PV matmul
        # (lhsT = one head slot, M=128) also produces the denominator
        # replicated on psum rows 0-63 (ctx on rows 64-127).  The ones
        # block comes FIRST so the reciprocal reads psum at partition
        # offset 0 — HW lowering drops nonzero partition offsets on the
        # custom-DVE reciprocal input.
        vsb = persist.tile([P, NSTILE, NH, 2 * D], BF16)
        wqT = persist.tile([P, NOTILE, H], BF16)
        wkT = persist.tile([P, NOTILE, H], BF16)
        wvT = persist.tile([P, NOTILE, H], BF16)
        wpT = persist.tile([P, NOTILE, H], BF16)
        cc2 = persist.tile([P, SPAD], BF16)            # cos^T stacked twice
        ss2 = persist.tile([P, SPAD], BF16)            # sin^T stacked, sign-baked
        bq_sb = persist.tile([P, NOTILE], F32)
        bvc = persist.tile([P, NOTILE], BF16)          # bv as [p, t] columns
        bp_row = persist.tile([1, H], BF16)
        bpe_row = persist.tile([1, H], BF16)           # bp' = Wp@bv + bp
        ones_row = persist.tile([1, P], BF16)
        ident = persist.tile([P, P], BF16)

        nc.vector.memset(ones_row, 1.0)

        # PE warmup: HAM flips to 8/8 after ~3.4us of sustained matmul
        # activity; transposes don't count.  One long accumulation group
        # (no inter-MM semaphores -> true back-to-back issue) of rank-1
        # updates keeps the PE busy ~4.5us so the prologue's real
        # matmuls run at 2.4 GHz.  Depends only on ones_row (DVE memset),
        # not on the gpsimd-built identity.
        wt = psum_sc.tile([P, P], F32, tag="sc", name="warm")
        NWARM = 40
        for i in range(NWARM):
            nc.tensor.matmul(wt, ones_row, ones_row,
                             start=(i == 0), stop=(i == NWARM - 1))

        from concourse.masks import make_identity
        make_identity(nc, ident)

        # ---------------- load helpers ----------------
        def load_bq():
            nc.gpsimd.dma_start(out=bq_sb, in_=bq_ext.rearrange("(t p) -> p t", p=P))

        _bias_tiles = {}

        def load_bias_posts():
            bstage2 = stage.tile([1, H], F32, tag="bp_stage", bufs=1)
            nc.gpsimd.dma_start(out=bstage2, in_=bp_ext.rearrange("(a h) -> a h", a=1))
            bvst = stage.tile([P, NOTILE], F32, tag="bias_stage", bufs=1)
            nc.gpsimd.dma_start(out=bvst, in_=bv_ext.rearrange("(t p) -> p t", p=P))
            _bias_tiles["bp"] = bstage2
            _bias_tiles["bv"] = bvst

        def load_bias_casts():
            # consumers are pt=5 only; emitted late so the casts never
            # head-of-line block the DVE/ACT queues in the prologue
            nc.scalar.copy(out=bp_row, in_=_bias_tiles["bp"])
            nc.vector.tensor_copy(out=bvc, in_=_bias_tiles["bv"])

        _cs = {}

        def _copy_on(eng, out, in_):
            if eng is nc.vector:
                nc.vector.tensor_copy(out=out, in_=in_)
            else:
                nc.scalar.copy(out=out, in_=in_)

        def _blocked(t):
            # [P, i, 64] view of the valid 64-wide runs in a [P, SPAD] tile
            return t[:, 0:10 * P].rearrange("p (i q) -> p i q", q=P)[:, :, 0:D]

        def _cblocked(t):
            # [P, i, 64] view of a compact [P, 11*64] staging tile
            return t[:, 0:10 * D].rearrange("p (i q) -> p i q", q=D)

        def load_sincos_posts():
            # [NROT, 64] f32 staged as [p, i, 64] in two DMAs per tensor
            # (rows 0:1280 as [128, 10, 64], tail 89 rows separately).
            # Posted from the GpSimd queue: 4 posts instead of v2's 22
            # sync-queue posts.
            for nm, src_ext in (("cos", cos_ext), ("sin", sin_ext)):
                cst = cstage.tile([P, NSTILE * D], F32, tag=f"cst_{nm}",
                                  name=f"cst_{nm}")
                nc.gpsimd.dma_start(
                    out=_cblocked(cst),
                    in_=src_ext[0:1280, :].rearrange("(i p) d -> p i d", p=P))
                nc.gpsimd.dma_start(
                    out=cst[0:NROT - 1280, 10 * D:10 * D + D],
                    in_=src_ext[1280:NROT, :])
                _cs[nm] = cst
            n_rtile = (NROT + P - 1) // P
            csb = cstage.tile([P, SPAD], BF16, tag="csb", name="csb")
            csT3 = cstage.tile([P, n_rtile, P], BF16, tag="csT3", name="csT3")
            nc.gpsimd.memset(csb, 0.0)
            _cs["csb"] = csb
            _cs["csT3"] = csT3

        def load_sincos_one(nm, dstT, cast_eng, scat_eng):
            cst, csb, csT3 = _cs[nm], _cs["csb"], _cs["csT3"]
            _copy_on(cast_eng, _blocked(csb), _cblocked(cst))
            _copy_on(cast_eng, csb[0:NROT - 1280, 10 * P:10 * P + D],
                     cst[0:NROT - 1280, 10 * D:10 * D + D])
            nc.scalar.dma_start_transpose(out=csT3, in_=csb)
            # wide scatter copies: [64, 10*128] + [64, 89] per half
            for half in range(2):
                hb = 64 * half
                _copy_on(scat_eng,
                         dstT[hb:hb + D, 0:10 * P].rearrange(
                             "p (i q) -> p i q", q=P),
                         csT3[0:D, 0:10, :])
                _copy_on(scat_eng, dstT[hb:hb + D, 10 * P:NROT],
                         csT3[0:D, 10, :NROT - 10 * P])

        def bake_sin_signs():
            for base in (0, 64):
                sl = slice(base, base + 32)
                nc.vector.tensor_scalar_mul(ss2[sl, :NROT], ss2[sl, :NROT], -1.0)

        _xevict = []

        def load_x_tile(st, pq):
            # transpose on the PE — the xbar path costs a 1.25us
            # scalar-queue post per tile plus sem-chained stage slots.
            # pq: queue for the DMA post (sync and gpsimd split the 11
            # loads so neither serializes the prologue).  Casts all on
            # DVE (its prologue queue is otherwise empty); evict
            # deferred one tile.
            s0, ssz = _stile(st)
            xs = stage.tile([P, H], F32, tag="x_stage", bufs=2, name=f"xs_{st}")
            xb = stage.tile([P, H], BF16, tag="x_stage_bf", bufs=2, name=f"xb_{st}")
            if ssz < P:
                nc.vector.memset(xb, 0.0)
            pq.dma_start(out=xs[:ssz], in_=x_ext[s0:s0 + ssz, :])
            nc.vector.tensor_copy(out=xb[:ssz], in_=xs[:ssz])
            xtp = psum_sc.tile([P, H], BF16, tag="sc", name=f"xtp_{st}")
            for kt in range(NOTILE):
                nc.tensor.transpose(xtp[:, kt * P:(kt + 1) * P],
                                    xb[:, kt * P:(kt + 1) * P], ident)
            _xevict.append((xtp, s0, st))
            if len(_xevict) > 1:
                flush_xevict(1)

        def flush_xevict(keep=0):
            while len(_xevict) > keep:
                xtp, s0, st = _xevict.pop(0)
                src = xtp.rearrange("p (k q) -> p k q", q=P)
                if st < 4 or st % 2 == 0:
                    nc.vector.tensor_copy(out=xT[:, :, s0:s0 + P], in_=src)
                else:
                    nc.scalar.copy(out=xT[:, :, s0:s0 + P], in_=src)

        def load_w_pe(w_ext, wT, r, key, ceng, eeng):
            # prologue path: transpose the weight row on the PE like an
            # x tile.  The xbar DMA_TRANSPOSE transfer takes ~6us and
            # its sem chains serialized the v4 prologue (wqT row 0 not
            # ready until ~32us); the PE is mostly idle here instead.
            ws = stage.tile([P, H], F32, tag=f"wst_{key}", bufs=1,
                            name=f"ws_{wT.name}_{r}")
            wb = stage.tile([P, H], BF16, tag=f"wstb_{key}", bufs=1,
                            name=f"wb_{wT.name}_{r}")
            nc.sync.dma_start(out=ws, in_=w_ext[r * P:(r + 1) * P, :])
            _copy_on(ceng, wb, ws)
            wtp = psum_sc.tile([P, H], BF16, tag="sc", name=f"wtp_{key}_{r}")
            for kt in range(NOTILE):
                nc.tensor.transpose(wtp[:, kt * P:(kt + 1) * P],
                                    wb[:, kt * P:(kt + 1) * P], ident)
            _copy_on(eeng, wT[:, :, r * P:(r + 1) * P],
                     wtp.rearrange("p (k q) -> p k q", q=P))

        def load_w_row(w_ext, wT, r, tq=None, ceng=None, key="", pq=None):
            # tq: engine queue for the xbar-transpose post (sync or
            # scalar only — hwdge).  ceng: engine for the f32->bf16
            # cast (ACT in the prologue while it's idle, DVE later).
            # Per-tensor stage tags: a wq cast must never WAR-wait a wk
            # xbar (the v3 trace showed such a wait head-of-line
            # blocking the DVE FIFO for 17us).
            ws = stage.tile([P, H], F32, tag=f"wst_{key}", bufs=1,
                            name=f"ws_{wT.name}_{r}")
            wb = stage.tile([P, H], BF16, tag=f"wstb_{key}", bufs=1,
                            name=f"wb_{wT.name}_{r}")
            (pq or nc.sync).dma_start(out=ws, in_=w_ext[r * P:(r + 1) * P, :])
            if ceng is nc.scalar:
                nc.scalar.copy(out=wb, in_=ws)
            else:
                nc.vector.tensor_copy(out=wb, in_=ws)
            (tq or nc.scalar).dma_start_transpose(
                out=wT[:, :, r * P:(r + 1) * P], in_=wb)

        # ---------------- projection emit-units ----------------
        _pj_live = {}

        def qkproj_half(wT, dst, ot, ci, bias, half, act_evict=False):
            # half 0: kts 0-2 (allocates psum); half 1: kts 3-5 + evict.
            # Split so paced filling can interleave at ~0.6us granularity.
            i0, ilen = IC[ci]
            key = (wT.name, ot, ci)
            if half == 0:
                _pj_live[key] = psum_pj.tile(
                    [P, 512], F32, tag="pj",
                    name=f"qk_{dst.name}_{ot}_{ci}")[:, :ilen]
            pj = _pj_live[key]
            for kt in range(3 * half, 3 * half + 3):
                nc.tensor.matmul(
                    pj, wT[:, kt, ot * P:(ot + 1) * P],
                    xT[:, kt, i0:i0 + ilen],
                    start=(kt == 0), stop=(kt == NOTILE - 1))
            if half == 1:
                del _pj_live[key]
                if bias:
                    if act_evict:
                        nc.scalar.add(dst[:, ot, i0:i0 + ilen], pj,
                                      bq_sb[:, ot:ot + 1])
                    else:
                        nc.vector.tensor_scalar_add(dst[:, ot, i0:i0 + ilen],
                                                    pj, bq_sb[:, ot:ot + 1])
                elif act_evict:
                    nc.scalar.copy(out=dst[:, ot, i0:i0 + ilen], in_=pj)
                else:
                    nc.vector.tensor_copy(out=dst[:, ot, i0:i0 + ilen], in_=pj)

        _rope_live = {}

        def rope_dma(dst, ot):
            # posted from the GpSimd queue: the Sync queue's transpose
            # posts carry long sem waits that would head-of-line block
            # these (v2's 13us mid-kernel PE stalls).
            rot = ropet.tile([P, NROT], BF16, tag="rot", name=f"rot_{dst.name}_{ot}")
            _rope_live[(dst.name, ot)] = rot
            sl = slice(PREFIX, PREFIX + NROT)
            for (dst0, src0) in ((0, 32), (32, 0), (64, 96), (96, 64)):
                nc.gpsimd.dma_start(
                    out=rot[dst0:dst0 + 32, :],
                    in_=dst[src0:src0 + 32, ot, sl])

        def rope_mul(dst, ot):
            # separate unit: the in-place mul WAR-waits on the rotate DMAs;
            # emitting it later keeps that wait off the DVE FIFO head
            rot = _rope_live.pop((dst.name, ot))
            sl = slice(PREFIX, PREFIX + NROT)
            nc.vector.tensor_mul(dst[:, ot, sl], dst[:, ot, sl], cc2[:, :NROT])
            nc.vector.tensor_mul(rot, rot, ss2[:, :NROT])
            nc.vector.tensor_add(dst[:, ot, sl], dst[:, ot, sl], rot)

        def vproj_st(pt, st):
            # no bias matmul: bv is folded into bp' (softmax rows sum
            # to 1, so it lands once per (s, d) via the out-proj bias)
            s0, ssz = _stile(st)
            pj = psum_pj.tile([P, 512], F32, tag="pj",
                              name=f"v_{pt}_{st}")[:, :P]
            for kt in range(NOTILE):
                nc.tensor.matmul(
                    pj[:ssz, :], xT[:, kt, s0:s0 + ssz],
                    wvT[:, kt, pt * P:(pt + 1) * P],
                    start=(kt == 0), stop=(kt == NOTILE - 1))
            nc.vector.tensor_copy(
                out=vsb[:ssz, st, 2 * pt:2 * pt + 2, D:2 * D],
                in_=pj[:ssz, :].rearrange("p (h d) -> p h d", d=D))

        def bp_eff_chunk(cj):
            # bp'[o] = sum_hd Wp[o, hd] bv[hd] + bp[o], on the PE.
            o0, on = ((0, 512), (512, 256))[cj]
            pj = psum_pj.tile([P, 512], F32, tag="pj",
                              name=f"bpe_{cj}")[:1, :on]
            for kt in range(NOTILE):
                nc.tensor.matmul(pj, bvc[:, kt:kt + 1], wpT[:, kt, o0:o0 + on],
                                 start=(kt == 0), stop=False)
            nc.tensor.matmul(pj, ones_row[:, 0:1], bp_row[:, o0:o0 + on],
                             start=False, stop=True)
            nc.scalar.copy(out=bpe_row[:, o0:o0 + on], in_=pj)

        def outproj_it(it):
            s0, ssz = _stile(it)
            ot_t = outst.tile([P, H], F32, tag="ostage", name=f"ost_{it}")
            for ci, (o0, on) in enumerate(((0, 512), (512, 256))):
                pj = psum_pj.tile([P, 512], F32, tag="pj",
                                  name=f"o_{it}_{ci}")[:, :on]
                for kt in range(NOTILE):
                    nc.tensor.matmul(
                        pj[:ssz, :], ctxT[:, kt, s0:s0 + ssz],
                        wpT[:, kt, o0:o0 + on],
                        start=(kt == 0), stop=False)
                nc.tensor.matmul(
                    pj[:ssz, :], ones_row[:, :ssz], bpe_row[:, o0:o0 + on],
                    start=False, stop=True)
                nc.scalar.copy(out=ot_t[:ssz, o0:o0 + on], in_=pj[:ssz, :])
            nc.sync.dma_start(out=out_ext[s0:s0 + ssz, :], in_=ot_t[:ssz])

        # ---------------- prologue emission ----------------
        # DMA posts first (transfers go async), then the x pipeline with
        # qkproj(0) chunk-halves interleaved between x-tile transposes.
        # Queue split: sync takes the w rows + x2-x6, gpsimd takes
        # x0/x1/x7-x10 + bq + sincos, so no queue's serial post stream
        # gates the pipeline.
        load_w_pe(wq_ext, wqT, 0, "q", nc.scalar, nc.scalar)
        load_x_tile(0, nc.sync)
        load_w_pe(wk_ext, wkT, 0, "k", nc.scalar, nc.scalar)
        load_x_tile(1, nc.sync)
        load_bq()
        load_bias_posts()
        load_w_pe(wv_ext, wvT, 0, "v", nc.scalar, nc.scalar)
        load_x_tile(2, nc.sync)
        load_x_tile(3, nc.sync)
        flush_xevict()

        def qk_unit(which, ci, half):
            if which == "q":
                qkproj_half(wqT, qT, 0, ci, True, half, act_evict=True)
            else:
                qkproj_half(wkT, kT, 0, ci, False, half, act_evict=True)

        qk_unit("q", 0, 0)
        load_x_tile(4, nc.sync)
        qk_unit("q", 0, 1)
        load_x_tile(5, nc.sync)
        qk_unit("k", 0, 0)
        load_x_tile(6, nc.sync)
        qk_unit("k", 0, 1)
        load_x_tile(7, nc.sync)
        flush_xevict()
        qk_unit("q", 1, 0)
        load_x_tile(8, nc.sync)
        qk_unit("q", 1, 1)
        load_x_tile(9, nc.sync)
        qk_unit("k", 1, 0)
        load_x_tile(10, nc.sync)
        flush_xevict()
        qk_unit("k", 1, 1)
        load_sincos_posts()
        load_sincos_one("cos", cc2, nc.vector, nc.vector)
        qk_unit("q", 2, 0)
        qk_unit("q", 2, 1)
        rope_dma(qT, 0)
        qk_unit("k", 2, 0)
        qk_unit("k", 2, 1)
        rope_dma(kT, 0)
        load_sincos_one("sin", ss2, nc.vector, nc.scalar)
        bake_sin_signs()
        load_bias_casts()
        # vsb ones after the rope posts so the (slow) memset doesn't
        # delay them on the GpSimd queue; first PV read is much later
        nc.gpsimd.memset(vsb[:, :, :, 0:D], 1.0)
        rope_mul(qT, 0)
        for st in range(NSTILE):
            if st == 1:
                rope_mul(kT, 0)
            vproj_st(0, st)

        # row 1 of each weight feeds proj(1), the attention(0) filler
        load_w_pe(wq_ext, wqT, 1, "q", nc.vector, nc.vector)
        load_w_pe(wk_ext, wkT, 1, "k", nc.vector, nc.vector)
        load_w_pe(wv_ext, wvT, 1, "v", nc.vector, nc.vector)

        def vhead_ap(jsz, jt, h):
            return vsb[:jsz, jt, h, :]

        def two_run_ap(t, rows, ilen):
            """[rows, 2, ilen] AP over a [P, 1024] tile: cols {0:ilen} and
            {512:512+ilen} — skips the unwritten hole when ilen < 512.
            For full-width chunks a flat 2D AP is equivalent and cheaper."""
            s = t[:rows, :]
            if ilen == 512:
                return s
            dims = [list(d) for d in s.ap]
            st = dims[-1][0]
            return bass.AP(tensor=s.tensor, offset=s.offset,
                           ap=[dims[0], [512 * st, 2], [st, ilen]])

        exp_f = mybir.ActivationFunctionType.Exp
        scaling = float(D) ** -0.5
        flush_norm = [lambda: None]

        for pt in range(NOTILE):
            # filler units: projections for pt+1 (for pt=4: only the first
            # 3 v-proj tiles — the rest fill attention(5, ic0) itself),
            # Wp loads during attention(0), out-proj row-tiles for pt=5.
            # just-in-time weight streaming: row pt+2 of Wq/Wk/Wv (feeds
            # proj(pt+2)) and one Wp row per pt — spread so no queue ever
            # sees a burst of weight traffic.
            fills = []
            if pt + 2 < NOTILE:
                for w_ext, wT, wk_ in ((wq_ext, wqT, "q"), (wk_ext, wkT, "k"),
                                       (wv_ext, wvT, "v")):
                    fills.append(lambda w_ext=w_ext, wT=wT, wk_=wk_:
                                 load_w_row(w_ext, wT, pt + 2, tq=nc.sync,
                                            key=wk_))
            if pt < NOTILE - 1:
                fills.append(lambda pt=pt: load_w_row(wp_ext, wpT, pt,
                                                      tq=nc.sync, key="p"))
                if pt == NOTILE - 2:
                    fills.append(lambda: load_w_row(wp_ext, wpT, NOTILE - 1,
                                                    tq=nc.sync, key="p"))
            if pt + 1 < NOTILE:
                np1 = pt + 1
                for ci3 in range(3):
                    for half in range(2):
                        fills.append(lambda ci3=ci3, half=half, np1=np1:
                                     qkproj_half(wqT, qT, np1, ci3, True, half))
                fills.append(lambda np1=np1: rope_dma(qT, np1))
                for ci3 in range(3):
                    for half in range(2):
                        fills.append(lambda ci3=ci3, half=half, np1=np1:
                                     qkproj_half(wkT, kT, np1, ci3, False, half))
                fills.append(lambda np1=np1: rope_dma(kT, np1))
                fills.append(lambda np1=np1: rope_mul(qT, np1))
                vmax = NSTILE if np1 < NOTILE - 1 else 3
                for st in range(vmax):
                    if st == 1:
                        fills.append(lambda np1=np1: rope_mul(kT, np1))
                    fills.append(lambda st=st, np1=np1: vproj_st(np1, st))
            else:
                # bp' on the PE once all wpT rows are resident
                fills.append(lambda: bp_eff_chunk(0))
                fills.append(lambda: bp_eff_chunk(1))
            # (for pt=5 the rest of v-proj(5) is emitted inline in the ic0
            # jt loop below — emission order must stay ahead of the PV
            # reads, since Tile tracks dependencies in trace order.)
            stage_fills = {}
            if pt == NOTILE - 1:
                # it 0-3 need ctxT i cols 0:512 (ready after ic0's
                # normalize); it 4-7 need cols up to 1024 (after ic1).
                stage_fills[1] = [lambda it=it: outproj_it(it) for it in range(4)]
                stage_fills[2] = [lambda it=it: outproj_it(it) for it in range(4, 8)]

            state = [0, 0]  # units emitted, paces done (of 39)

            def pace():
                state[1] += 1
                tgt = min(len(fills), -(-len(fills) * state[1] // 45))
                while state[0] < tgt:
                    fills[state[0]]()
                    state[0] += 1

            for ci, (i0, ilen) in enumerate(IC):
                if pt == NOTILE - 1:
                    # out-proj fills read ctxT; the pending normalize must
                    # be emitted before they are
                    flush_norm[0]()
                if ci in stage_fills:
                    fills.extend(stage_fills[ci])
                pvbox = [None]

                def emit_pv(item, pvbox=pvbox, ilen=ilen, pt=pt, ci=ci):
                    if pvbox[0] is None:
                        pvbox[0] = psum_pv.tile([P, 1024], F32, tag="pv",
                                                name=f"pv_{pt}_{ci}")
                    pv = pvbox[0]
                    pes, pjt, pjsz = item
                    for hh in range(2):
                        nc.tensor.matmul(
                            pv[:, 512 * hh:512 * hh + ilen],
                            vhead_ap(pjsz, pjt, 2 * pt + hh),
                            pes[:pjsz, 512 * hh:512 * hh + ilen],
                            start=(pjt == 0), stop=(pjt == NSTILE - 1))

                pending = []
                for jt in range(NSTILE):
                    j0, jsz = _stile(jt)
                    sc = psum_sc.tile([P, 1024], F32, tag="sc",
                                      name=f"sc_{pt}_{ci}_{jt}")
                    for hh in range(2):
                        hb = 64 * hh
                        nc.tensor.matmul(
                            sc[:jsz, 512 * hh:512 * hh + ilen],
                            kT[hb:hb + 64, pt, j0:j0 + jsz],
                            qT[hb:hb + 64, pt, i0:i0 + ilen],
                            start=True, stop=True)
                    es = es_pool.tile([P, 1024], BF16, tag="es",
                                      name=f"es_{pt}_{ci}_{jt}")
                    nc.scalar.activation(out=two_run_ap(es, jsz, ilen),
                                         in_=two_run_ap(sc, jsz, ilen),
                                         func=exp_f, scale=scaling)
                    if jt == 1:
                        # lazy normalize of the previous chunk: emitted
                        # after this chunk's first exps so it never
                        # head-of-line blocks the DVE FIFO
                        flush_norm[0]()
                    if pt == NOTILE - 1 and ci == 0 and jt + 3 < NSTILE:
                        vproj_st(pt, jt + 3)
                    else:
                        pace()
                    if len(pending) >= 2:
                        emit_pv(pending.pop(0))
                    pending.append((es, jt, jsz))
                for item in pending:
                    pace()
                    emit_pv(item)

                def norm(pv=pvbox[0], ilen=ilen, i0=i0, pt=pt, ci=ci):
                    # denominator is replicated on psum rows 0-63
                    rec = rec_pool.tile([D, 1024], F32, tag="rec",
                                        name=f"rec_{pt}_{ci}")
                    nc.vector.reciprocal_approx_fast(
                        out=two_run_ap(rec, D, ilen),
                        in_=two_run_ap(pv, D, ilen))
                    for hh in range(2):
                        nc.vector.tensor_mul(
                            ctxT[64 * hh:64 * hh + 64, pt, i0:i0 + ilen],
                            pv[64:128, 512 * hh:512 * hh + ilen],
                            rec[0:64, 512 * hh:512 * hh + ilen])

                def mk_flush(fn):
                    def f():
                        flush_norm[0] = lambda: None
                        fn()
                    return f

                flush_norm[0] = mk_flush(norm)
                pace()
                pace()
            while state[0] < len(fills):
                fills[state[0]]()
                state[0] += 1

        # ---------------- output projection tail ----------------
        flush_norm[0]()
        for it in range(8, NSTILE):
            outproj_it(it)


_NC_CACHE = None


def get_nc():
    global _NC_CACHE
    if _NC_CACHE is None:
        nc = bacc.Bacc(None, target_bir_lowering=False, debug=False)
        _NC_CACHE = build_kernel(nc)
    return _NC_CACHE


def kernel(**inputs):
    from concourse.bass_utils import run_bass_kernel_spmd

    nc = get_nc()
    names = ["hidden_states", "sin", "cos", "Wq", "bq", "Wk", "Wv", "bv", "Wp", "bp"]
    arrs = {k: np.ascontiguousarray(np.asarray(inputs[k], dtype=np.float32))
            for k in names}
    in_maps = []
    for b in range(B):
        m = {k: arrs[k] for k in names if k != "hidden_states"}
        m["hidden_states"] = np.ascontiguousarray(arrs["hidden_states"][b])
        in_maps.append(m)
    res = run_bass_kernel_spmd(nc, in_maps, core_ids=list(range(B)))
    out = np.stack([res.results[b]["out"] for b in range(B)], axis=0)
    return out.astype(np.float32)


if __name__ == "__main__":
    nc = get_nc()
    print("built ok")


# revision 13
# speedup vs baseline: 1.0898x; 1.0898x over previous
"""Dinov3 self-attention Bass kernel for TRN2.

Sharding: data-parallel over batch. B=8 batch elements -> 8 NeuronCores,
one full attention per core, weights replicated. No collectives.

v3 design (v2 + trace-driven fixes):
  - v2 traits kept: qT/kT/ctxT head-transposed layouts, psum-packed
    two-head score chunks with one wide exp, ones-column denominator
    trick, on-chip normalize, paced projection fillers.
  - PE warmup burst: ~36 dummy matmuls at t~2us flip the HAM clock
    gate to 8/8 before the real prologue matmuls issue (transposes
    don't register as PE activity, so without this the whole prologue
    runs at 1.2 GHz).
  - prologue is emission-interleaved: qkproj(0) chunk-halves between
    x-tile transposes, casts/evicts alternating DVE/ACT, so the PE
    never head-of-line blocks on a single elementwise queue.
  - v-bias folded into the output projection: softmax rows sum to 1,
    so ctx(v + bv) = ctx(v) + bv and  out = ctx@Wp.T + (Wp@bv + bp).
    bp' = Wp@bv + bp is computed once on the PE (12 tiny matmuls);
    the 66 per-tile v-bias matmuls + LDWEIGHTS disappear.
  - rope rotate DMAs, bias loads and sincos staging post from the
    (otherwise idle) GpSimd queue: in v2 they sat on the Sync queue
    behind DMA_TRANSPOSE posts whose semaphore waits head-of-line
    blocked them ~50us, starving DVE and stalling the PE 13us at a
    time mid-kernel.
  - sincos staged with 4 large DMAs (was 22 small ones) and 8 wide
    scatter copies (was 44).
Engines: PE matmuls; ACT exp + prologue casts/evicts; DVE everything
elementwise; GpSimd queue posts rope/bias/sincos DMAs; Sync queue
posts x/w loads, w transposes, out stores.
"""

import contextlib
import sys

import numpy as np

sys.path.insert(0, "/opt/trn_rl_repo")

import concourse.bacc as bacc
import concourse.bass as bass
import concourse.tile as tile
from concourse import mybir

S = 1374
H = 768
NH = 12
D = 64
NROT = 1369
PREFIX = S - NROT  # 5
B = 8

P = 128
NSTILE = (S + P - 1) // P  # 11 s-tiles, last has 94 rows
NOTILE = H // P  # 6 head pairs
SPAD = NSTILE * P  # 1408
IC = ((0, 512), (512, 512), (1024, 350))  # i-chunks, each <= 1 psum bank

F32 = mybir.dt.float32
BF16 = mybir.dt.bfloat16


def _stile(i):
    start = i * P
    return start, min(P, S - start)


def build_kernel(nc):
    x_ext = nc.declare_dram_parameter("hidden_states", [S, H], F32, isOutput=False)
    sin_ext = nc.declare_dram_parameter("sin", [NROT, D], F32, isOutput=False)
    cos_ext = nc.declare_dram_parameter("cos", [NROT, D], F32, isOutput=False)
    wq_ext = nc.declare_dram_parameter("Wq", [H, H], F32, isOutput=False)
    bq_ext = nc.declare_dram_parameter("bq", [H], F32, isOutput=False)
    wk_ext = nc.declare_dram_parameter("Wk", [H, H], F32, isOutput=False)
    wv_ext = nc.declare_dram_parameter("Wv", [H, H], F32, isOutput=False)
    bv_ext = nc.declare_dram_parameter("bv", [H], F32, isOutput=False)
    wp_ext = nc.declare_dram_parameter("Wp", [H, H], F32, isOutput=False)
    bp_ext = nc.declare_dram_parameter("bp", [H], F32, isOutput=False)
    out_ext = nc.declare_dram_parameter("out", [S, H], F32, isOutput=True)

    with tile.TileContext(nc) as tc:
        _body(tc, x_ext, sin_ext, cos_ext, wq_ext, bq_ext, wk_ext,
              wv_ext, bv_ext, wp_ext, bp_ext, out_ext)
    nc.compile()
    return nc


def _body(tc, x_ext, sin_ext, cos_ext, wq_ext, bq_ext, wk_ext, wv_ext,
          bv_ext, wp_ext, bp_ext, out_ext):
    nc = tc.nc

    with contextlib.ExitStack() as ctx:
        persist = ctx.enter_context(tc.tile_pool(name="persist", bufs=1))
        psum_sc = ctx.enter_context(tc.tile_pool(name="psum_sc", bufs=2, space="PSUM"))
        psum_pv = ctx.enter_context(tc.tile_pool(name="psum_pv", bufs=1, space="PSUM"))
        psum_pj = ctx.enter_context(tc.tile_pool(name="psum_pj", bufs=2, space="PSUM"))
        es_pool = ctx.enter_context(tc.tile_pool(name="es_pool", bufs=3))
        rec_pool = ctx.enter_context(tc.tile_pool(name="rec_pool", bufs=1))
        ropet = ctx.enter_context(tc.tile_pool(name="ropet", bufs=2))
        stage = ctx.enter_context(tc.tile_pool(name="stage", bufs=2))
        cstage = ctx.enter_context(tc.tile_pool(name="cstage", bufs=1))
        outst = ctx.enter_context(tc.tile_pool(name="outst", bufs=2))

        xT = persist.tile([P, NOTILE, SPAD], BF16)     # xT[p, t, s] = x[s, 128t+p]
        qT = persist.tile([P, NOTILE, SPAD], BF16)     # roped q, [(hh,d), pt, s]
        kT = persist.tile([P, NOTILE, SPAD], BF16)
        ctxT = persist.tile([P, NOTILE, SPAD], BF16)   # normalized ctx^T
        # per head: 64 ones columns + 64 v columns, so every PV matmul
        # (lhsT = one head slot, M=128) also produces the denominator
        # replicated on psum rows 0-63 (ctx on rows 64-127).  The ones
        # block comes FIRST so the reciprocal reads psum at partition
        # offset 0 — HW lowering drops nonzero partition offsets on the
        # custom-DVE reciprocal input.
        vsb = persist.tile([P, NSTILE, NH, 2 * D], BF16)
        wqT = persist.tile([P, NOTILE, H], BF16)
        wkT = persist.tile([P, NOTILE, H], BF16)
        wvT = persist.tile([P, NOTILE, H], BF16)
        wpT = persist.tile([P, NOTILE, H], BF16)
        cc2 = persist.tile([P, SPAD], BF16)            # cos^T stacked twice
        ss2 = persist.tile([P, SPAD], BF16)            # sin^T stacked, sign-baked
        bq_sb = persist.tile([P, NOTILE], F32)
        bvc = persist.tile([P, NOTILE], BF16)          # bv as [p, t] columns
        bp_row = persist.tile([1, H], BF16)
        bpe_row = persist.tile([1, H], BF16)           # bp' = Wp@bv + bp
        ones_row = persist.tile([1, P], BF16)
        ident = persist.tile([P, P], BF16)

        nc.vector.memset(ones_row, 1.0)

        # PE warmup: HAM flips to 8/8 after ~3.4us of sustained matmul
        # activity; transposes don't count.  One long accumulation group
        # (no inter-MM semaphores -> true back-to-back issue) of rank-1
        # updates keeps the PE busy ~4.5us so the prologue's real
        # matmuls run at 2.4 GHz.  Depends only on ones_row (DVE memset),
        # not on the gpsimd-built identity.
        wt = psum_sc.tile([P, P], F32, tag="sc", name="warm")
        NWARM = 40
        for i in range(NWARM):
            nc.tensor.matmul(wt, ones_row, ones_row,
                             start=(i == 0), stop=(i == NWARM - 1))

        from concourse.masks import make_identity
        make_identity(nc, ident)

        # ---------------- load helpers ----------------
        def load_bq():
            nc.gpsimd.dma_start(out=bq_sb, in_=bq_ext.rearrange("(t p) -> p t", p=P))

        _bias_tiles = {}

        def load_bias_posts():
            bstage2 = stage.tile([1, H], F32, tag="bp_stage", bufs=1)
            nc.gpsimd.dma_start(out=bstage2, in_=bp_ext.rearrange("(a h) -> a h", a=1))
            bvst = stage.tile([P, NOTILE], F32, tag="bias_stage", bufs=1)
            nc.gpsimd.dma_start(out=bvst, in_=bv_ext.rearrange("(t p) -> p t", p=P))
            _bias_tiles["bp"] = bstage2
            _bias_tiles["bv"] = bvst

        def load_bias_casts():
            # consumers are pt=5 only; emitted late so the casts never
            # head-of-line block the DVE/ACT queues in the prologue
            nc.scalar.copy(out=bp_row, in_=_bias_tiles["bp"])
            nc.vector.tensor_copy(out=bvc, in_=_bias_tiles["bv"])

        _cs = {}

        def _copy_on(eng, out, in_):
            if eng is nc.vector:
                nc.vector.tensor_copy(out=out, in_=in_)
            else:
                nc.scalar.copy(out=out, in_=in_)

        def _blocked(t):
            # [P, i, 64] view of the valid 64-wide runs in a [P, SPAD] tile
            return t[:, 0:10 * P].rearrange("p (i q) -> p i q", q=P)[:, :, 0:D]

        def _cblocked(t):
            # [P, i, 64] view of a compact [P, 11*64] staging tile
            return t[:, 0:10 * D].rearrange("p (i q) -> p i q", q=D)

        def load_sincos_posts():
            # [NROT, 64] f32 staged compactly as [p, i, 64]; posted from
            # the Sync queue after the x loads (hwdge, predictable).
            for nm, src_ext in (("cos", cos_ext), ("sin", sin_ext)):
                cst = cstage.tile([P, NSTILE * D], F32, tag=f"cst_{nm}",
                                  name=f"cst_{nm}")
                nc.sync.dma_start(
                    out=_cblocked(cst),
                    in_=src_ext[0:1280, :].rearrange("(i p) d -> p i d", p=P))
                nc.sync.dma_start(
                    out=cst[0:NROT - 1280, 10 * D:10 * D + D],
                    in_=src_ext[1280:NROT, :])
                _cs[nm] = cst

        def load_sincos_pe(nm, dstT):
            # transpose on the PE (the xbar path's ~6us transfers and
            # sem chains kept landing mid-queue and stalling DVE).
            # ctp[d, i*128+p] = src[i*128+p, d]; evict into both halves
            # of dstT, with the rotate_half signs baked for sin.
            cst = _cs[nm]
            csb = cstage.tile([P, NSTILE * D], BF16, tag=f"csb_{nm}",
                              name=f"csb_{nm}")
            nc.vector.memset(csb, 0.0)
            nc.vector.tensor_copy(out=csb[:, 0:10 * D], in_=cst[:, 0:10 * D])
            nc.vector.tensor_copy(
                out=csb[0:NROT - 1280, 10 * D:10 * D + D],
                in_=cst[0:NROT - 1280, 10 * D:10 * D + D])
            ctp = psum_sc.tile([D, SPAD], BF16, tag="sc", name=f"ctp_{nm}")
            for i in range(NSTILE):
                nc.tensor.transpose(ctp[:, i * P:(i + 1) * P],
                                    csb[:, i * D:(i + 1) * D], ident)
            if nm == "cos":
                nc.vector.tensor_copy(out=dstT[0:D, :NROT], in_=ctp[0:D, :NROT])
                nc.scalar.copy(out=dstT[D:2 * D, :NROT], in_=ctp[0:D, :NROT])
            else:
                for hb in (0, D):
                    nc.vector.tensor_scalar_mul(
                        dstT[hb:hb + 32, :NROT], ctp[0:32, :NROT], -1.0)
                    nc.scalar.copy(out=dstT[hb + 32:hb + D, :NROT],
                                   in_=ctp[32:D, :NROT])

        _xevict = []

        def load_x_tile(st, pq):
            # transpose on the PE — the xbar path costs a 1.25us
            # scalar-queue post per tile plus sem-chained stage slots.
            # pq: queue for the DMA post (sync and gpsimd split the 11
            # loads so neither serializes the prologue).  Casts all on
            # DVE (its prologue queue is otherwise empty); evict
            # deferred one tile.
            s0, ssz = _stile(st)
            xs = stage.tile([P, H], F32, tag="x_stage", bufs=2, name=f"xs_{st}")
            xb = stage.tile([P, H], BF16, tag="x_stage_bf", bufs=2, name=f"xb_{st}")
            if ssz < P:
                nc.vector.memset(xb, 0.0)
            pq.dma_start(out=xs[:ssz], in_=x_ext[s0:s0 + ssz, :])
            nc.vector.tensor_copy(out=xb[:ssz], in_=xs[:ssz])
            xtp = psum_sc.tile([P, H], BF16, tag="sc", name=f"xtp_{st}")
            for kt in range(NOTILE):
                nc.tensor.transpose(xtp[:, kt * P:(kt + 1) * P],
                                    xb[:, kt * P:(kt + 1) * P], ident)
            _xevict.append((xtp, s0, st))
            if len(_xevict) > 1:
                flush_xevict(1)

        def flush_xevict(keep=0):
            while len(_xevict) > keep:
                xtp, s0, st = _xevict.pop(0)
                src = xtp.rearrange("p (k q) -> p k q", q=P)
                if st < 4 or st % 2 == 0:
                    nc.vector.tensor_copy(out=xT[:, :, s0:s0 + P], in_=src)
                else:
                    nc.scalar.copy(out=xT[:, :, s0:s0 + P], in_=src)

        def load_w_pe(w_ext, wT, r, key, ceng, eeng):
            # prologue path: transpose the weight row on the PE like an
            # x tile.  The xbar DMA_TRANSPOSE transfer takes ~6us and
            # its sem chains serialized the v4 prologue (wqT row 0 not
            # ready until ~32us); the PE is mostly idle here instead.
            ws = stage.tile([P, H], F32, tag=f"wst_{key}", bufs=1,
                            name=f"ws_{wT.name}_{r}")
            wb = stage.tile([P, H], BF16, tag=f"wstb_{key}", bufs=1,
                            name=f"wb_{wT.name}_{r}")
            nc.sync.dma_start(out=ws, in_=w_ext[r * P:(r + 1) * P, :])
            _copy_on(ceng, wb, ws)
            wtp = psum_sc.tile([P, H], BF16, tag="sc", name=f"wtp_{key}_{r}")
            for kt in range(NOTILE):
                nc.tensor.transpose(wtp[:, kt * P:(kt + 1) * P],
                                    wb[:, kt * P:(kt + 1) * P], ident)
            _copy_on(eeng, wT[:, :, r * P:(r + 1) * P],
                     wtp.rearrange("p (k q) -> p k q", q=P))

        def load_w_row(w_ext, wT, r, tq=None, ceng=None, key="", pq=None):
            # tq: engine queue for the xbar-transpose post (sync or
            # scalar only — hwdge).  ceng: engine for the f32->bf16
            # cast (ACT in the prologue while it's idle, DVE later).
            # Per-tensor stage tags: a wq cast must never WAR-wait a wk
            # xbar (the v3 trace showed such a wait head-of-line
            # blocking the DVE FIFO for 17us).
            ws = stage.tile([P, H], F32, tag=f"wst_{key}", bufs=1,
                            name=f"ws_{wT.name}_{r}")
            wb = stage.tile([P, H], BF16, tag=f"wstb_{key}", bufs=1,
                            name=f"wb_{wT.name}_{r}")
            (pq or nc.sync).dma_start(out=ws, in_=w_ext[r * P:(r + 1) * P, :])
            if ceng is nc.scalar:
                nc.scalar.copy(out=wb, in_=ws)
            else:
                nc.vector.tensor_copy(out=wb, in_=ws)
            (tq or nc.scalar).dma_start_transpose(
                out=wT[:, :, r * P:(r + 1) * P], in_=wb)

        # ---------------- projection emit-units ----------------
        _pj_live = {}

        def qkproj_half(wT, dst, ot, ci, bias, half, act_evict=False):
            # half 0: kts 0-2 (allocates psum); half 1: kts 3-5 + evict.
            # Split so paced filling can interleave at ~0.6us granularity.
            i0, ilen = IC[ci]
            key = (wT.name, ot, ci)
            if half == 0:
                _pj_live[key] = psum_pj.tile(
                    [P, 512], F32, tag="pj",
                    name=f"qk_{dst.name}_{ot}_{ci}")[:, :ilen]
            pj = _pj_live[key]
            for kt in range(3 * half, 3 * half + 3):
                nc.tensor.matmul(
                    pj, wT[:, kt, ot * P:(ot + 1) * P],
                    xT[:, kt, i0:i0 + ilen],
                    start=(kt == 0), stop=(kt == NOTILE - 1))
            if half == 1:
                del _pj_live[key]
                if bias:
                    if act_evict:
                        nc.scalar.add(dst[:, ot, i0:i0 + ilen], pj,
                                      bq_sb[:, ot:ot + 1])
                    else:
                        nc.vector.tensor_scalar_add(dst[:, ot, i0:i0 + ilen],
                                                    pj, bq_sb[:, ot:ot + 1])
                elif act_evict:
                    nc.scalar.copy(out=dst[:, ot, i0:i0 + ilen], in_=pj)
                else:
                    nc.vector.tensor_copy(out=dst[:, ot, i0:i0 + ilen], in_=pj)

        _rope_live = {}

        def rope_dma(dst, ot, pq=None):
            # steady state: posted from the GpSimd queue (the Sync
            # queue's transpose posts carry long sem waits that would
            # head-of-line block these — v2's 13us mid-kernel PE
            # stalls).  Prologue: posted from Sync, which is idle then.
            rot = ropet.tile([P, NROT], BF16, tag="rot", name=f"rot_{dst.name}_{ot}")
            _rope_live[(dst.name, ot)] = rot
            sl = slice(PREFIX, PREFIX + NROT)
            for (dst0, src0) in ((0, 32), (32, 0), (64, 96), (96, 64)):
                (pq or nc.gpsimd).dma_start(
                    out=rot[dst0:dst0 + 32, :],
                    in_=dst[src0:src0 + 32, ot, sl])

        def rope_mul(dst, ot):
            # separate unit: the in-place mul WAR-waits on the rotate DMAs;
            # emitting it later keeps that wait off the DVE FIFO head
            rot = _rope_live.pop((dst.name, ot))
            sl = slice(PREFIX, PREFIX + NROT)
            nc.vector.tensor_mul(dst[:, ot, sl], dst[:, ot, sl], cc2[:, :NROT])
            nc.vector.tensor_mul(rot, rot, ss2[:, :NROT])
            nc.vector.tensor_add(dst[:, ot, sl], dst[:, ot, sl], rot)

        def vproj_st(pt, st):
            # no bias matmul: bv is folded into bp' (softmax rows sum
            # to 1, so it lands once per (s, d) via the out-proj bias)
            s0, ssz = _stile(st)
            pj = psum_pj.tile([P, 512], F32, tag="pj",
                              name=f"v_{pt}_{st}")[:, :P]
            for kt in range(NOTILE):
                nc.tensor.matmul(
                    pj[:ssz, :], xT[:, kt, s0:s0 + ssz],
                    wvT[:, kt, pt * P:(pt + 1) * P],
                    start=(kt == 0), stop=(kt == NOTILE - 1))
            nc.vector.tensor_copy(
                out=vsb[:ssz, st, 2 * pt:2 * pt + 2, D:2 * D],
                in_=pj[:ssz, :].rearrange("p (h d) -> p h d", d=D))

        def bp_eff_chunk(cj):
            # bp'[o] = sum_hd Wp[o, hd] bv[hd] + bp[o], on the PE.
            o0, on = ((0, 512), (512, 256))[cj]
            pj = psum_pj.tile([P, 512], F32, tag="pj",
                              name=f"bpe_{cj}")[:1, :on]
            for kt in range(NOTILE):
                nc.tensor.matmul(pj, bvc[:, kt:kt + 1], wpT[:, kt, o0:o0 + on],
                                 start=(kt == 0), stop=False)
            nc.tensor.matmul(pj, ones_row[:, 0:1], bp_row[:, o0:o0 + on],
                             start=False, stop=True)
            nc.scalar.copy(out=bpe_row[:, o0:o0 + on], in_=pj)

        def outproj_it(it):
            s0, ssz = _stile(it)
            ot_t = outst.tile([P, H], F32, tag="ostage", name=f"ost_{it}")
            for ci, (o0, on) in enumerate(((0, 512), (512, 256))):
                pj = psum_pj.tile([P, 512], F32, tag="pj",
                                  name=f"o_{it}_{ci}")[:, :on]
                for kt in range(NOTILE):
                    nc.tensor.matmul(
                        pj[:ssz, :], ctxT[:, kt, s0:s0 + ssz],
                        wpT[:, kt, o0:o0 + on],
                        start=(kt == 0), stop=False)
                nc.tensor.matmul(
                    pj[:ssz, :], ones_row[:, :ssz], bpe_row[:, o0:o0 + on],
                    start=False, stop=True)
                nc.scalar.copy(out=ot_t[:ssz, o0:o0 + on], in_=pj[:ssz, :])
            nc.sync.dma_start(out=out_ext[s0:s0 + ssz, :], in_=ot_t[:ssz])

        # ---------------- prologue emission ----------------
        # DMA posts first (transfers go async), then the x pipeline with
        # qkproj(0) chunk-halves interleaved between x-tile transposes.
        # Queue split: sync takes the w rows + x2-x6, gpsimd takes
        # x0/x1/x7-x10 + bq + sincos, so no queue's serial post stream
        # gates the pipeline.
        load_w_pe(wq_ext, wqT, 0, "q", nc.scalar, nc.scalar)
        load_x_tile(0, nc.sync)
        load_w_pe(wk_ext, wkT, 0, "k", nc.scalar, nc.scalar)
        load_x_tile(1, nc.sync)
        load_bq()
        load_bias_posts()
        load_w_pe(wv_ext, wvT, 0, "v", nc.scalar, nc.scalar)
        load_x_tile(2, nc.sync)
        load_x_tile(3, nc.sync)
        flush_xevict()

        def qk_unit(which, ci, half):
            if which == "q":
                qkproj_half(wqT, qT, 0, ci, True, half, act_evict=True)
            else:
                qkproj_half(wkT, kT, 0, ci, False, half, act_evict=True)

        qk_unit("q", 0, 0)
        load_x_tile(4, nc.sync)
        qk_unit("q", 0, 1)
        load_x_tile(5, nc.sync)
        qk_unit("k", 0, 0)
        load_x_tile(6, nc.sync)
        qk_unit("k", 0, 1)
        load_x_tile(7, nc.sync)
        flush_xevict()
        qk_unit("q", 1, 0)
        load_x_tile(8, nc.sync)
        qk_unit("q", 1, 1)
        load_x_tile(9, nc.sync)
        qk_unit("k", 1, 0)
        load_x_tile(10, nc.sync)
        flush_xevict()
        qk_unit("k", 1, 1)
        load_sincos_posts()
        qk_unit("q", 2, 0)
        qk_unit("q", 2, 1)
        rope_dma(qT, 0, pq=nc.sync)
        qk_unit("k", 2, 0)
        qk_unit("k", 2, 1)
        rope_dma(kT, 0, pq=nc.sync)
        load_sincos_pe("cos", cc2)
        load_sincos_pe("sin", ss2)
        load_bias_casts()
        # vsb ones after the rope posts so the (slow) memset doesn't
        # delay them on the GpSimd queue; first PV read is much later
        nc.gpsimd.memset(vsb[:, :, :, 0:D], 1.0)
        rope_mul(qT, 0)
        for st in range(NSTILE):
            if st == 1:
                rope_mul(kT, 0)
            vproj_st(0, st)

        # row 1 of each weight feeds proj(1), the attention(0) filler
        load_w_pe(wq_ext, wqT, 1, "q", nc.vector, nc.vector)
        load_w_pe(wk_ext, wkT, 1, "k", nc.vector, nc.vector)
        load_w_pe(wv_ext, wvT, 1, "v", nc.vector, nc.vector)

        def vhead_ap(jsz, jt, h):
            return vsb[:jsz, jt, h, :]

        def two_run_ap(t, rows, ilen):
            """[rows, 2, ilen] AP over a [P, 1024] tile: cols {0:ilen} and
            {512:512+ilen} — skips the unwritten hole when ilen < 512.
            For full-width chunks a flat 2D AP is equivalent and cheaper."""
            s = t[:rows, :]
            if ilen == 512:
                return s
            dims = [list(d) for d in s.ap]
            st = dims[-1][0]
            return bass.AP(tensor=s.tensor, offset=s.offset,
                           ap=[dims[0], [512 * st, 2], [st, ilen]])

        exp_f = mybir.ActivationFunctionType.Exp
        scaling = float(D) ** -0.5
        flush_norm = [lambda: None]

        for pt in range(NOTILE):
            # filler units: projections for pt+1 (for pt=4: only the first
            # 3 v-proj tiles — the rest fill attention(5, ic0) itself),
            # Wp loads during attention(0), out-proj row-tiles for pt=5.
            # just-in-time weight streaming: row pt+2 of Wq/Wk/Wv (feeds
            # proj(pt+2)) and one Wp row per pt — spread so no queue ever
            # sees a burst of weight traffic.
            fills = []
            if pt + 2 < NOTILE:
                for w_ext, wT, wk_ in ((wq_ext, wqT, "q"), (wk_ext, wkT, "k"),
                                       (wv_ext, wvT, "v")):
                    fills.append(lambda w_ext=w_ext, wT=wT, wk_=wk_:
                                 load_w_row(w_ext, wT, pt + 2, tq=nc.sync,
                                            key=wk_))
            if pt < NOTILE - 1:
                fills.append(lambda pt=pt: load_w_row(wp_ext, wpT, pt,
                                                      tq=nc.sync, key="p"))
                if pt == NOTILE - 2:
                    fills.append(lambda: load_w_row(wp_ext, wpT, NOTILE - 1,
                                                    tq=nc.sync, key="p"))
            if pt + 1 < NOTILE:
                np1 = pt + 1
                for ci3 in range(3):
                    for half in range(2):
                        fills.append(lambda ci3=ci3, half=half, np1=np1:
                                     qkproj_half(wqT, qT, np1, ci3, True, half))
                fills.append(lambda np1=np1: rope_dma(qT, np1))
                for ci3 in range(3):
                    for half in range(2):
                        fills.append(lambda ci3=ci3, half=half, np1=np1:
                                     qkproj_half(wkT, kT, np1, ci3, False, half))
                fills.append(lambda np1=np1: rope_dma(kT, np1))
                fills.append(lambda np1=np1: rope_mul(qT, np1))
                vmax = NSTILE if np1 < NOTILE - 1 else 3
                for st in range(vmax):
                    if st == 1:
                        fills.append(lambda np1=np1: rope_mul(kT, np1))
                    fills.append(lambda st=st, np1=np1: vproj_st(np1, st))
            else:
                # bp' on the PE once all wpT rows are resident
                fills.append(lambda: bp_eff_chunk(0))
                fills.append(lambda: bp_eff_chunk(1))
            # (for pt=5 the rest of v-proj(5) is emitted inline in the ic0
            # jt loop below — emission order must stay ahead of the PV
            # reads, since Tile tracks dependencies in trace order.)
            stage_fills = {}
            if pt == NOTILE - 1:
                # it 0-3 need ctxT i cols 0:512 (ready after ic0's
                # normalize); it 4-7 need cols up to 1024 (after ic1).
                stage_fills[1] = [lambda it=it: outproj_it(it) for it in range(4)]
                stage_fills[2] = [lambda it=it: outproj_it(it) for it in range(4, 8)]

            state = [0, 0]  # units emitted, paces done (of 39)

            def pace():
                state[1] += 1
                tgt = min(len(fills), -(-len(fills) * state[1] // 45))
                while state[0] < tgt:
                    fills[state[0]]()
                    state[0] += 1

            for ci, (i0, ilen) in enumerate(IC):
                if pt == NOTILE - 1:
                    # out-proj fills read ctxT; the pending normalize must
                    # be emitted before they are
                    flush_norm[0]()
                if ci in stage_fills:
                    fills.extend(stage_fills[ci])
                pvbox = [None]

                def emit_pv(item, pvbox=pvbox, ilen=ilen, pt=pt, ci=ci):
                    if pvbox[0] is None:
                        pvbox[0] = psum_pv.tile([P, 1024], F32, tag="pv",
                                                name=f"pv_{pt}_{ci}")
                    pv = pvbox[0]
                    pes, pjt, pjsz = item
                    for hh in range(2):
                        nc.tensor.matmul(
                            pv[:, 512 * hh:512 * hh + ilen],
                            vhead_ap(pjsz, pjt, 2 * pt + hh),
                            pes[:pjsz, 512 * hh:512 * hh + ilen],
                            start=(pjt == 0), stop=(pjt == NSTILE - 1))

                pending = []
                for jt in range(NSTILE):
                    j0, jsz = _stile(jt)
                    sc = psum_sc.tile([P, 1024], F32, tag="sc",
                                      name=f"sc_{pt}_{ci}_{jt}")
                    for hh in range(2):
                        hb = 64 * hh
                        nc.tensor.matmul(
                            sc[:jsz, 512 * hh:512 * hh + ilen],
                            kT[hb:hb + 64, pt, j0:j0 + jsz],
                            qT[hb:hb + 64, pt, i0:i0 + ilen],
                            start=True, stop=True)
                    es = es_pool.tile([P, 1024], BF16, tag="es",
                                      name=f"es_{pt}_{ci}_{jt}")
                    nc.scalar.activation(out=two_run_ap(es, jsz, ilen),
                                         in_=two_run_ap(sc, jsz, ilen),
                                         func=exp_f, scale=scaling)
                    if jt == 1:
                        # lazy normalize of the previous chunk: emitted
                        # after this chunk's first exps so it never
                        # head-of-line blocks the DVE FIFO
                        flush_norm[0]()
                    if pt == NOTILE - 1 and ci == 0 and jt + 3 < NSTILE:
                        vproj_st(pt, jt + 3)
                    else:
                        pace()
                    if len(pending) >= 2:
                        emit_pv(pending.pop(0))
                    pending.append((es, jt, jsz))
                for item in pending:
                    pace()
                    emit_pv(item)

                def norm(pv=pvbox[0], ilen=ilen, i0=i0, pt=pt, ci=ci):
                    # denominator is replicated on psum rows 0-63
                    rec = rec_pool.tile([D, 1024], F32, tag="rec",
                                        name=f"rec_{pt}_{ci}")
                    nc.vector.reciprocal_approx_fast(
                        out=two_run_ap(rec, D, ilen),
                        in_=two_run_ap(pv, D, ilen))
                    for hh in range(2):
                        nc.vector.tensor_mul(
                            ctxT[64 * hh:64 * hh + 64, pt, i0:i0 + ilen],
                            pv[64:128, 512 * hh:512 * hh + ilen],
                            rec[0:64, 512 * hh:512 * hh + ilen])

                def mk_flush(fn):
                    def f():
                        flush_norm[0] = lambda: None
                        fn()
                    return f

                flush_norm[0] = mk_flush(norm)
                pace()
                pace()
            while state[0] < len(fills):
                fills[state[0]]()
                state[0] += 1

        # ---------------- output projection tail ----------------
        flush_norm[0]()
        for it in range(8, NSTILE):
            outproj_it(it)


_NC_CACHE = None


def get_nc():
    global _NC_CACHE
    if _NC_CACHE is None:
        nc = bacc.Bacc(None, target_bir_lowering=False, debug=False)
        _NC_CACHE = build_kernel(nc)
    return _NC_CACHE


def kernel(**inputs):
    from concourse.bass_utils import run_bass_kernel_spmd

    nc = get_nc()
    names = ["hidden_states", "sin", "cos", "Wq", "bq", "Wk", "Wv", "bv", "Wp", "bp"]
    arrs = {k: np.ascontiguousarray(np.asarray(inputs[k], dtype=np.float32))
            for k in names}
    in_maps = []
    for b in range(B):
        m = {k: arrs[k] for k in names if k != "hidden_states"}
        m["hidden_states"] = np.ascontiguousarray(arrs["hidden_states"][b])
        in_maps.append(m)
    res = run_bass_kernel_spmd(nc, in_maps, core_ids=list(range(B)))
    out = np.stack([res.results[b]["out"] for b in range(B)], axis=0)
    return out.astype(np.float32)


if __name__ == "__main__":
    nc = get_nc()
    print("built ok")
